# revision 1
# baseline (speedup 1.0000x reference)
"""Trainium2 Bass kernel for nn_AttnBlock (sparse GQA attention block).

Sharding: 8 cores = batch(2) x head-group(4). Each core handles one batch's
sequence with 4 q-heads + their shared kv-head (GQA group), computes its
partial output projection; host sums the 4 group partials per batch and adds
the residual.

Device kernel (per core, SPMD): x^T (bf16, host-pretransposed) -> token rms
stats (square + ones-matmul + rsqrt via ln/exp) -> qkv projection
(token-major psum, rms scale folded into the V evict; q/k rms scale folded
ahead of rope, which is linear) -> rope -> PE transposes to feature-major
q/k -> block-sparse attention over the host-derived k-tile structure
(scores computed transposed [k,q]; additive masks added via identity
matmuls; exp without max subtraction, safe because |s| <= 8 for rms-normed
q,k; denominators via all-ones matmul broadcast + DVE reciprocal)
-> PV into head-paired partition layout -> output projection -> partial
out^T (bf16) to DRAM.
"""

import sys
from contextlib import ExitStack

try:
    import concourse.bass  # noqa: F401  (provided by the axon site tree)
except ImportError:
    sys.path.insert(0, "/opt/trn_rl_repo")

import numpy as np
import ml_dtypes

import concourse.bass as bass
import concourse.tile as tile
import concourse.mybir as mybir
from concourse.masks import make_identity

F32 = mybir.dt.float32
BF16 = mybir.dt.bfloat16
BF = ml_dtypes.bfloat16

B, L, D = 2, 2048, 1024
HEADS, KV_HEADS, DH = 16, 4, 64
WINDOW = 1024
NEG = -1e30
EPS = 1.1920929e-07
NT = L // 128          # 16 token tiles
ND = D // 128          # 8 d tiles
NG = 4                 # head groups (= cores per batch)
SCALE = 1.0 / np.sqrt(DH)


def split_multi_waits(nc):
    """This environment's walrus supports only ONE sync wait per instruction.
    Split each multi-wait instruction into single-wait NoOps inserted just
    before it (same engine; per-engine execution is in-order, so consecutive
    single waits are equivalent to one multi-wait)."""
    for func in nc.m.functions:
        for block in func.blocks:
            new_list = []
            for inst in block.instructions:
                si = inst.sync_info
                if si is not None and len(si.on_wait) > 1:
                    waits = list(si.on_wait)
                    for w in waits[:-1]:
                        new_list.append(mybir.InstNoOp(
                            name=f"waitsplit-{nc.next_id()}",
                            engine=inst.engine,
                            sync_info=mybir.SyncInfo(on_wait=[w], on_update=[]),
                            text_hint="waitsplit", bass_nofuse=True))
                    inst.sync_info = mybir.SyncInfo(
                        on_wait=[waits[-1]], on_update=list(si.on_update))
                if si is not None and len(si.on_update) > 1:
                    ups = list(inst.sync_info.on_update)
                    inst.sync_info = mybir.SyncInfo(
                        on_wait=list(inst.sync_info.on_wait), on_update=[ups[0]])
                    new_list.append(inst)
                    for u in ups[1:]:
                        new_list.append(mybir.InstNoOp(
                            name=f"updsplit-{nc.next_id()}",
                            engine=inst.engine,
                            sync_info=mybir.SyncInfo(on_wait=[], on_update=[u]),
                            text_hint="updsplit", bass_nofuse=True))
                    continue
                new_list.append(inst)
            block.instructions[:] = new_list


# ---------------------------------------------------------------- host plan

def plan_structure(reset_mask: np.ndarray):
    """Derive the union block-sparse structure and per-batch additive masks."""
    lo = np.zeros((B, L), np.int64)
    idx = np.arange(L)
    for b in range(B):
        r = np.where(np.asarray(reset_mask[b], bool), idx, 0)
        last_reset = np.maximum.accumulate(r)
        lo[b] = np.maximum(last_reset, idx - (WINDOW - 1))

    kts = []
    widths = []
    for qt in range(NT):
        kt_min = min(int(lo[b, 128 * qt] // 128) for b in range(B))
        kts.append(list(range(kt_min, qt + 1)))
        ws = []
        qs = np.arange(128 * qt, 128 * qt + 128)
        for kt in kts[qt]:
            if kt == qt:
                ws.append(128)
            else:
                cross = max(int((lo[b, qs] < 128 * (kt + 1)).sum()) for b in range(B))
                ws.append(min(128, max(32, -(-cross // 32) * 32)))
        widths.append(ws)

    pairs = [(qt, kt) for qt in range(NT) for kt in kts[qt]]
    masks = np.zeros((B, len(pairs), 128, 128), np.float32)
    kk = idx[:128]
    for b in range(B):
        for i, (qt, kt) in enumerate(pairs):
            k = 128 * kt + kk[:, None]             # [128,1] global k
            q = 128 * qt + kk[None, :]             # [1,128] global q
            valid = (k >= lo[b, 128 * qt:128 * qt + 128][None, :]) & (k <= q)
            masks[b, i] = np.where(valid, 0.0, NEG)
    return kts, widths, masks


# ------------------------------------------------------------ device build

def build_program(kts, widths):
    pairs = [(qt, kt) for qt in range(NT) for kt in kts[qt]]
    pair_idx = {p: i for i, p in enumerate(pairs)}
    NP = len(pairs)
    # ragged score strips: older (partial) k-tiles contribute 4*W columns,
    # the diagonal tile 4*128. Score psum strips are built in k-tile groups
    # of <= 1024 f32 per partition (2 banks); exp evicts each group into one
    # per-qt pT buffer at its global offset. The fixed-seed structure always
    # has one group per q tile.
    goffs = []             # goffs[qt][ki] = global offset in pT
    groups = []            # groups[qt] = list of (kis, {ki: local_off}, sf)
    SFmax = 0              # max psum strip (per group)
    PTmax = 0              # max total pT width (per qt)
    for qt in range(NT):
        ws = widths[qt]
        nk = len(ws)
        go, o = {}, 0
        for ki in range(nk - 1, -1, -1):   # diagonal tile first
            go[ki] = o
            o += 4 * ws[ki]
        goffs.append(go)
        PTmax = max(PTmax, o)
        gs, cur, cursz = [], [], 0
        for ki in range(nk - 1, -1, -1):
            w4 = 4 * ws[ki]
            if cur and cursz + w4 > 1024:
                gs.append(cur)
                cur, cursz = [], 0
            cur.append(ki)
            cursz += w4
        gs.append(cur)
        fin = []
        for kis in gs:
            offs = {}
            oo = 0
            for ki in kis:
                offs[ki] = oo
                oo += 4 * widths[qt][ki]
            fin.append((kis, offs, oo))
            SFmax = max(SFmax, oo)
        groups.append(fin)

    nc = bass.Bass("TRN2", target_bir_lowering=False, debug=False, num_devices=8)
    ap_xT = nc.dram_tensor("xT", [ND, 128, L], BF16, kind="ExternalInput").ap()
    ap_wqkvT = nc.dram_tensor("wqkvT", [ND, 128, 384], BF16, kind="ExternalInput").ap()
    ap_woutP = nc.dram_tensor("woutP", [2, 128, D], BF16, kind="ExternalInput").ap()
    ap_cosF = nc.dram_tensor("cosF", [NT, 128, DH], BF16, kind="ExternalInput").ap()
    ap_sinF2 = nc.dram_tensor("sinF2", [NT, 128, DH], BF16, kind="ExternalInput").ap()
    ap_masks = nc.dram_tensor("masks", [NP, 128, 128], BF16, kind="ExternalInput").ap()
    ap_outT = nc.dram_tensor("outT", [D, L], BF16, kind="ExternalOutput").ap()

    with tile.TileContext(nc) as tc, ExitStack() as ctx:
        csts = ctx.enter_context(tc.tile_pool(name="consts", bufs=1))
        big = ctx.enter_context(tc.tile_pool(name="big", bufs=1))
        dramp = ctx.enter_context(tc.tile_pool(name="dram", bufs=1, space="DRAM"))

        ident = csts.tile([128, 128], BF16, tag="ident")
        allones = csts.tile([128, 128], BF16, tag="allones")
        eps_c = csts.tile([128, 1], F32, tag="eps_c")
        make_identity(nc, ident)
        nc.gpsimd.memset(allones, 1.0)
        nc.gpsimd.memset(eps_c, EPS)

        xT_sb = big.tile([128, ND, L], BF16, tag="xT")
        for dj in range(ND):
            nc.sync.dma_start(out=xT_sb[:, dj, :], in_=ap_xT[dj])
        wqkv_sb = csts.tile([128, ND, 384], BF16, tag="wqkv")
        nc.sync.dma_start(out=wqkv_sb, in_=ap_wqkvT.rearrange("n p f -> p n f"))
        cos_sb = csts.tile([128, NT, DH], BF16, tag="cos")
        nc.sync.dma_start(out=cos_sb, in_=ap_cosF.rearrange("n p f -> p n f"))
        sin_sb = csts.tile([128, NT, DH], BF16, tag="sin")
        nc.sync.dma_start(out=sin_sb, in_=ap_sinF2.rearrange("n p f -> p n f"))
        mask_sb = csts.tile([128, NP, 128], BF16, tag="mask")
        nc.sync.dma_start(out=mask_sb, in_=ap_masks.rearrange("n p f -> p n f"))
        wout_sb = csts.tile([128, 2, D], BF16, tag="wout")
        nc.sync.dma_start(out=wout_sb, in_=ap_woutP.rearrange("n p f -> p n f"))

        qkv_raw = big.tile([128, NT, 6, DH], BF16, tag="qkv_raw")  # 0:5 q/k, 5 v
        qk_rot = big.tile([128, NT, 5, DH], BF16, tag="qk_rot")
        qTp = big.tile([128, NT, 2, 128], BF16, tag="qTp")  # [.., pair, ..]: heads
        # (2*pair, 2*pair+1) stacked on partition halves
        kvT = big.tile([128, NT, 128], BF16, tag="kvT")     # kT in BOTH partition halves
        yTn2 = big.tile([128, 2, NT, 128], BF16, tag="yTn2")
        s_cols = big.tile([128, NT], F32, tag="s_cols")
        ms_qk = big.tile([128, NT, 5], F32, tag="ms_qk")
        ln_qk = big.tile([128, NT, 5], F32, tag="ln_qk")
        s_qk = big.tile([128, NT, 5], F32, tag="s_qk")
        ms_sb = big.tile([1, L], F32, tag="ms_sb")
        s_ms = big.tile([128, NT], F32, tag="s_ms")
        scratch_dram = dramp.tile([L], F32)

        def sb_ap(t, offset_elems, dims):
            return bass.AP(tensor=t.tensor, offset=t.offset + offset_elems,
                           ap=[t.ap[0]] + dims)

        # ---- phase A: qkv projection + per-tile stats + rope + transpose --
        with tc.tile_pool(name="x2p", bufs=2) as x2p, \
             tc.tile_pool(name="msps", bufs=1, space="PSUM") as msps, \
             tc.tile_pool(name="qkvps", bufs=2, space="PSUM") as qkvps, \
             tc.tile_pool(name="trps", bufs=2, space="PSUM") as trps, \
             tc.tile_pool(name="sqp", bufs=2) as sqp, \
             tc.tile_pool(name="qsc", bufs=2) as qsc:
            half = DH // 2
            for ti in range(NT):
                qkv_ps = qkvps.tile([128, 384], F32)
                for dj in range(ND):
                    nc.tensor.matmul(
                        qkv_ps, xT_sb[:, dj, 128 * ti:128 * ti + 128],
                        wqkv_sb[:, dj, :], start=(dj == 0), stop=(dj == ND - 1))
                # single evict (q/k/v raw, unscaled) on ACT (idle here)
                nc.scalar.activation(
                    out=qkv_raw[:, ti, :, :],
                    in_=qkv_ps.rearrange("p (h d) -> p h d", d=DH),
                    func=mybir.ActivationFunctionType.Copy)
                # q/k mean-square stats (gpsimd square, DVE reduce)
                sq = sqp.tile([128, 5, DH], BF16, tag="sq")
                nc.gpsimd.tensor_mul(sq, qkv_raw[:, ti, 0:5, :],
                                     qkv_raw[:, ti, 0:5, :])
                nc.vector.tensor_reduce(out=ms_qk[:, ti, :], in_=sq,
                                        axis=mybir.AxisListType.X,
                                        op=mybir.AluOpType.add)
                if ti % 4 == 3:
                    # rsqrt(ms/DH + eps) = exp(-0.5 * ln(ms/DH + eps)) for a
                    # group of 4 tiles (small ops; avoids a global barrier)
                    g = ti - 3
                    nc.scalar.activation(
                        out=ln_qk[:, g:ti + 1, :], in_=ms_qk[:, g:ti + 1, :],
                        func=mybir.ActivationFunctionType.Ln,
                        scale=1.0 / DH, bias=eps_c)
                    nc.scalar.activation(
                        out=s_qk[:, g:ti + 1, :], in_=ln_qk[:, g:ti + 1, :],
                        func=mybir.ActivationFunctionType.Exp, scale=-0.5)

            for ti in range(NT):
                qks = qsc.tile([128, 5, DH], BF16, tag="qks")
                for h in range(5):
                    nc.vector.tensor_scalar_mul(qks[:, h, :],
                                                qkv_raw[:, ti, h, :],
                                                s_qk[:, ti, h:h + 1])
                cos_b = sb_ap(cos_sb, ti * DH, [[0, 5], [1, DH]])
                sin_b = sb_ap(sin_sb, ti * DH, [[0, 5], [1, DH]])
                qswap = sb_ap(qks, half, [[DH, 5], [-half, 2], [1, half]])
                ra = qsc.tile([128, 5, DH], BF16, tag="ra")
                rb = qsc.tile([128, 5, DH], BF16, tag="rb")
                nc.vector.tensor_mul(ra, qks, cos_b)
                nc.gpsimd.tensor_mul(rb, qswap, sin_b)
                nc.vector.tensor_add(qk_rot[:, ti, :, :], ra, rb)

                # transposes: (q0,q1) -> qTp0, (q2,q3) -> qTp1, and kT into
                # both partition halves of kvT
                tr2 = trps.tile([128, 2, 128], BF16, tag="tr")
                nc.tensor.transpose(tr2[:, 0, :], qk_rot[:, ti, 0:2, :], ident)
                nc.tensor.transpose(tr2[:, 1, :], qk_rot[:, ti, 2:4, :], ident)
                nc.scalar.activation(out=qTp[:, ti, :, :], in_=tr2,
                                     func=mybir.ActivationFunctionType.Copy)
                trk = trps.tile([64, 128], BF16, tag="tr")
                nc.tensor.transpose(trk, qk_rot[:, ti, 4, :], ident)
                nc.vector.tensor_copy(kvT[0:64, ti, :], trk)
                trk2 = trps.tile([128, 128], BF16, tag="tr")
                nc.tensor.transpose(trk2, qk_rot[:, ti, 3:5, :], ident)
                nc.scalar.activation(out=kvT[64:128, ti, :], in_=trk2[64:128, :],
                                     func=mybir.ActivationFunctionType.Copy)

            # ---- token rms stats for the V scale (off critical path) -----
            ms_ps = msps.tile([1, 4, 512], F32)
            for dj in range(ND):
                x2 = x2p.tile([128, L], BF16, tag="x2")
                if dj < 6:
                    nc.vector.tensor_mul(x2, xT_sb[:, dj, :], xT_sb[:, dj, :])
                else:
                    nc.gpsimd.tensor_mul(x2[:, 0:1024], xT_sb[:, dj, 0:1024],
                                         xT_sb[:, dj, 0:1024])
                    nc.gpsimd.tensor_mul(x2[:, 1024:2048], xT_sb[:, dj, 1024:2048],
                                         xT_sb[:, dj, 1024:2048])
                for c in range(4):
                    nc.tensor.matmul(
                        ms_ps[:, c, :], allones[:, 0:1], x2[:, 512 * c:512 * c + 512],
                        start=(dj == 0), stop=(dj == ND - 1))
            nc.scalar.activation(out=ms_sb.rearrange("p (a b) -> p a b", a=4),
                                 in_=ms_ps,
                                 func=mybir.ActivationFunctionType.Copy)
            nc.sync.dma_start(out=scratch_dram, in_=ms_sb)
            nc.sync.dma_start(out=s_ms,
                              in_=scratch_dram.rearrange("(c p) -> p c", p=128))
            nc.scalar.activation(out=s_ms, in_=s_ms,
                                 func=mybir.ActivationFunctionType.Ln,
                                 scale=1.0 / D, bias=eps_c)
            nc.scalar.activation(out=s_cols, in_=s_ms,
                                 func=mybir.ActivationFunctionType.Exp, scale=-0.5)
            # scale V in place
            for ti in range(NT):
                nc.vector.tensor_scalar_mul(qkv_raw[:, ti, 5, :],
                                            qkv_raw[:, ti, 5, :],
                                            s_cols[:, ti:ti + 1])

        # ---- phase B: attention per q tile + interleaved output proj ----
        sps_b, den_b, y2_b, ops_b, ptp_b, rp_b = 1, 1, 2, 2, 3, 3
        with tc.tile_pool(name="sps", bufs=sps_b, space="PSUM") as sps, \
             tc.tile_pool(name="denps", bufs=den_b, space="PSUM") as denps, \
             tc.tile_pool(name="y2ps", bufs=y2_b, space="PSUM") as y2ps, \
             tc.tile_pool(name="ops", bufs=ops_b, space="PSUM") as ops, \
             tc.tile_pool(name="ptp", bufs=ptp_b) as ptp, \
             tc.tile_pool(name="rp", bufs=rp_b) as rp, \
             tc.tile_pool(name="osb", bufs=2) as osb:
            outT_cols = ap_outT.rearrange("(n p) l -> p n l", p=128)
            for qt in range(NT):
                nk = len(kts[qt])
                ws = widths[qt]
                go = goffs[qt]
                pT = ptp.tile([128, PTmax], BF16, tag="pT")
                for kis, offs, sf in groups[qt]:
                    s_ps = sps.tile([128, SFmax], F32, tag="s_ps")
                    for ki in kis:
                        kt, w = kts[qt][ki], ws[ki]
                        for h in range(4):
                            base = 64 * (h % 2)
                            rhs = qTp[base:base + 64, qt, h // 2, 0:w]
                            reg = s_ps[:, offs[ki] + h * w: offs[ki] + (h + 1) * w]
                            nc.tensor.matmul(reg, kvT[base:base + 64, kt, :], rhs,
                                             start=True, stop=False)
                            nc.tensor.matmul(reg, ident,
                                             mask_sb[:, pair_idx[(qt, kt)], 0:w],
                                             start=False, stop=True)
                    gbase = go[kis[0]]
                    nc.scalar.activation(out=pT[:, gbase:gbase + sf],
                                         in_=s_ps[:, 0:sf],
                                         func=mybir.ActivationFunctionType.Exp,
                                         scale=SCALE)
                den_ps = denps.tile([128, 512], F32, tag="den")
                first = True
                for ki in range(nk - 1, -1, -1):
                    kt, w = kts[qt][ki], ws[ki]
                    off = go[ki]
                    last = ki == 0
                    if w == 128:
                        nc.tensor.matmul(den_ps, allones,
                                         pT[:, off:off + 512],
                                         start=first, stop=last)
                        first = False
                    else:
                        for h in range(4):
                            nc.tensor.matmul(
                                den_ps[:, 128 * h:128 * h + w], allones,
                                pT[:, off + h * w: off + (h + 1) * w],
                                start=first, stop=(last and h == 3))
                            first = False
                r_t = rp.tile([128, 512], F32, tag="r_t")
                nc.vector.reciprocal(out=r_t, in_=den_ps)
                y2_ps = y2ps.tile([128, 2, 128], F32, tag="y2")
                for h in range(4):
                    pbase = 64 * (h % 2)
                    out_reg = y2_ps[pbase:pbase + 64, h // 2, :]
                    for ki in range(nk - 1, -1, -1):
                        kt, w = kts[qt][ki], ws[ki]
                        off = go[ki]
                        nc.tensor.matmul(
                            out_reg[:, 0:w] if w < 128 else out_reg,
                            qkv_raw[:, kt, 5, :],
                            pT[:, off + h * w: off + (h + 1) * w],
                            start=(ki == nk - 1), stop=(ki == 0))
                r_even = bass.AP(tensor=r_t.tensor, offset=r_t.offset,
                                 ap=[[r_t.ap[0][0], 64], [256, 2], [1, 128]])
                r_odd = bass.AP(tensor=r_t.tensor, offset=r_t.offset + 128,
                                ap=[[r_t.ap[0][0], 64], [256, 2], [1, 128]])
                nc.vector.tensor_mul(yTn2[0:64, :, qt, :], y2_ps[0:64, :, :], r_even)
                nc.vector.tensor_mul(yTn2[64:128, :, qt, :], y2_ps[64:128, :, :], r_odd)

                # interleave the output projection for finished q-chunks
                if qt % 4 == 3:
                    c = qt // 4
                    o_col = osb.tile([128, ND, 512], BF16, tag="o_col")
                    for dt in range(ND):
                        o_ps = ops.tile([128, 512], F32, tag="o_ps")
                        for pair in range(2):
                            nc.tensor.matmul(
                                o_ps,
                                wout_sb[:, pair, 128 * dt:128 * dt + 128],
                                yTn2[:, pair, 4 * c:4 * c + 4, :].rearrange(
                                    "p a b -> p (a b)"),
                                start=(pair == 0), stop=(pair == 1))
                        if dt % 2 == 0:
                            nc.vector.tensor_copy(o_col[:, dt, :], o_ps)
                        else:
                            nc.scalar.activation(
                                out=o_col[:, dt, :], in_=o_ps,
                                func=mybir.ActivationFunctionType.Copy)
                    nc.sync.dma_start(out=outT_cols[:, :, 512 * c:512 * c + 512],
                                      in_=o_col)

    return nc


# ------------------------------------------------------------- host driver

_COS_SIN = None


def _cos_sin():
    global _COS_SIN
    if _COS_SIN is None:
        half = DH // 2
        inv_freq = 1.0 / (10000.0 ** (np.arange(half, dtype=np.float32) / half))
        f = np.outer(np.arange(L, dtype=np.float32), inv_freq)
        cosF = np.concatenate([np.cos(f), np.cos(f)], -1).astype(BF).reshape(NT, 128, DH)
        sinF2 = np.concatenate([-np.sin(f), np.sin(f)], -1).astype(BF).reshape(NT, 128, DH)
        _COS_SIN = (cosF, sinF2)
    return _COS_SIN


def make_core_inputs(x, w_qkv, w_out, masks, b, g):
    xT = np.ascontiguousarray(x[b].T).astype(BF).reshape(ND, 128, L)
    wg = np.concatenate([
        w_qkv[256 * g:256 * g + 256],
        w_qkv[1024 + 64 * g:1024 + 64 * g + 64],
        w_qkv[1280 + 64 * g:1280 + 64 * g + 64]], 0)        # [384, 1024]
    wqkvT = np.ascontiguousarray(wg.T).astype(BF).reshape(ND, 128, 384)
    woutP = np.stack([
        np.ascontiguousarray(w_out[:, 256 * g + 128 * p:256 * g + 128 * p + 128].T)
        for p in range(2)]).astype(BF)                       # [2, 128, 1024]
    cosF, sinF2 = _cos_sin()
    return {
        "xT": xT, "wqkvT": wqkvT, "woutP": woutP,
        "cosF": cosF, "sinF2": sinF2,
        "masks": np.ascontiguousarray(masks[b]).astype(BF),
    }


_PROGRAM_CACHE = {}


def get_program(kts, widths):
    key = (tuple(tuple(k) for k in kts), tuple(tuple(w) for w in widths))
    if key not in _PROGRAM_CACHE:
        _PROGRAM_CACHE[key] = build_program(kts, widths)
    return _PROGRAM_CACHE[key]


def kernel(x, w_qkv, w_out, reset_mask):
    x = np.asarray(x, np.float32)
    w_qkv = np.asarray(w_qkv, np.float32)
    w_out = np.asarray(w_out, np.float32)
    reset_mask = np.asarray(reset_mask)

    kts, widths, masks = plan_structure(reset_mask)
    nc = get_program(kts, widths)
    if not getattr(nc, "_waitsplit_done", False):
        split_multi_waits(nc)
        nc._waitsplit_done = True

    in_maps = [make_core_inputs(x, w_qkv, w_out, masks, b, g)
               for b in range(B) for g in range(NG)]
    from concourse import bass_utils
    res = bass_utils.run_bass_kernel_spmd(nc, in_maps, core_ids=list(range(8)))

    out = x.copy()
    core = 0
    for b in range(B):
        acc = np.zeros((D, L), np.float32)
        for g in range(NG):
            acc += res.results[core]["outT"].astype(np.float32)
            core += 1
        out[b] += acc.T
    return out

